# revision 29
# baseline (speedup 1.0000x reference)
"""Causal attention (B=1, H=16, S=2048, D=64, fp32 in/out) on 8 trn2 cores.

Sharding: 2 heads per core (fully head-parallel); inputs split / outputs
concatenated on host.

v5 design — PE-array tiling packs both heads into every matmul slot:
  - host hands each core qT/kT bf16 [128, S] pre-transposed (rows h*64+d)
    and v bf16 [HPC, S, D]; no on-chip transposes or casts;
  - QK, per (i-block 512, j-tile 128): TWO row-tiled matmuls run
    CONCURRENTLY on the PE (h0 in array rows 0:64, h1 in rows 64:128 —
    K=64 each), writing dots_h0/dots_h1 to the two banks of a
    [128, 1024] PSUM tile: one N-cycle slot computes both heads;
  - exp: ONE instruction per j-tile covers both heads via a 3-D AP over
    the two banks; routed ACT (exact, scale folded) vs DVE (Schraudolph
    fast-exp: bf16_bits = int16(dots*A + B), single fused tensor_scalar)
    by a greedy balancer; diagonal tiles always exact + gpsimd
    affine_select causal zeroing (end-to-end rel_err ~5e-3 vs 2e-2 budget);
  - AV, per j-tile: TWO col-tiled matmuls run concurrently (v_h0 ->
    acc[0:64], v_h1 -> acc[64:128], M=64 each, K=128): one N-cycle slot;
    the softmax denominators (the ones-column of older revisions doesn't
    fit col tiling) come from 4-way col-tiled ones-matmuls (M=1 at
    partitions 0/32/64/96, two j-tiles x two heads per slot);
  - epilogue: acc + den PSUM banks copied raw to SBUF and DMA'd out
    UNNORMALIZED; the host transposes and divides in numpy.
PE work per core: QK 17.4K + AV 17.4K + den 8.7K ~= 44K cycles; ACT/DVE
carry ~10us of exp each; everything else hides under the PE stream.
"""

import os

import numpy as np

import concourse.bass as bass
import concourse.mybir as mybir
import concourse.tile as tile
from concourse.vector_clock import ScopedClock

B, H, S, D = 1, 16, 2048, 64
NCORES = 8
HPC = H // NCORES  # heads per core
ST = S // 128  # seq tiles of 128
IB = 512  # i-block width
NB = S // IB  # i-blocks
JPB = IB // 128  # j-tiles per i-block (4)
SCALE = float(D) ** -0.5

F32 = mybir.dt.float32
BF16 = mybir.dt.bfloat16
I16 = mybir.dt.int16

# Schraudolph fast-exp constants (bf16-bits variant, scale folded in):
#   bf16_bits(exp(scale*x)) ~= int16(x * EXP_A + EXP_B)
EXP_C = 330000.0  # sawtooth-centering offset (tuned end-to-end)
EXP_A = SCALE * (2.0**23 / np.log(2.0)) / 65536.0
EXP_B = (127.0 * 2.0**23 - EXP_C) / 65536.0 + 0.25  # +0.25: round/trunc-robust

# greedy exp-router cost model (ns): per-column rate + per-instruction setup
ACT_RATE, ACT_OVH = 0.833, 280.0
DVE_RATE, DVE_OVH = 1.042, 170.0


# --------------------------------------------------------------------------
# Workarounds for the walrus in this container: an instruction may carry at
# most ONE sync-wait command ("Too many sync wait commands" in setupSyncWait
# otherwise).  (a) split the TileContext final drain into one drain per
# semaphore, (b) split any scheduled instruction with >1 wait by hoisting
# extra waits onto preceding same-engine NoOps.
# --------------------------------------------------------------------------
_MAXW = 1


def _split_drain_and_barrier(self, tick_clock, wait_clock):
    vclock = tick_clock.global_clock
    pending = [(proc, vclock[proc]) for proc in range(len(vclock)) if vclock[proc] > 0]
    engines = [self.nc.sync, self.nc.vector, self.nc.scalar, self.nc.gpsimd,
               self.nc.tensor]
    for i in range(0, len(pending), _MAXW):
        d = engines[(i // _MAXW) % len(engines)].drain()
        sc = ScopedClock()
        for proc, t in pending[i : i + _MAXW]:
            sc.require_at_least(None, proc, t)
        wait_clock.add_sem_waits(d.ins, sc)
    self.nc.all_engine_barrier()
    popped = self.nc._tile_sem_poison_stack.pop()
    assert popped is self._sem_poison
    self.nc.clear_and_free_semaphores(list(self.sems.allocated().values()))
    self.nc.all_engine_barrier()


_orig_lower = tile.TileContext._lower_ordered_insts


def _split_waits_lower(self, ordered):
    import bass_rust

    for bbname in list(ordered.keys()):
        out = []
        for inst in ordered[bbname]:
            si = inst.sync_info
            if si is not None and len(si.on_wait) > _MAXW:
                waits = list(si.on_wait)
                extra, keep = waits[:-_MAXW], waits[-_MAXW:]
                for i in range(0, len(extra), _MAXW):
                    nop = mybir.InstNoOp(
                        name=f"{inst.name}-wsplit{i}", ins=[], outs=[]
                    )
                    nop.engine = inst.engine
                    nop.sync_info = bass_rust.SyncInfo(
                        on_wait=extra[i : i + _MAXW], on_update=[]
                    )
                    out.append(nop)
                inst.sync_info = bass_rust.SyncInfo(
                    on_wait=keep, on_update=list(si.on_update)
                )
            out.append(inst)
        ordered[bbname] = out
    return _orig_lower(self, ordered)


class _PatchedTileContext(tile.TileContext):
    _drain_and_barrier = _split_drain_and_barrier
    _lower_ordered_insts = _split_waits_lower


# --------------------------------------------------------------------------
# Kernel build
# --------------------------------------------------------------------------


def build_nc(fastexp=True):
    SKEWJ = int(os.environ.get("K_SKEWJ", "3"))  # j-tile QK->AV lookahead
    DOTS_BUFS = int(os.environ.get("K_DOTS", "3"))
    ATTN_BUFS = int(os.environ.get("K_ATTN", "8"))
    NWARM = int(os.environ.get("K_WARM", "0"))
    if os.environ.get("K_FASTEXP", "1") == "0":
        fastexp = False

    nc = bass.Bass("TRN2")
    qT = nc.dram_tensor("qT", [128, S], BF16, kind="ExternalInput")
    kT = nc.dram_tensor("kT", [128, S], BF16, kind="ExternalInput")
    v = nc.dram_tensor("v", [HPC, S, D], BF16, kind="ExternalInput")
    oN = nc.dram_tensor("oN", [NB, 128, IB], BF16, kind="ExternalOutput")
    oD = nc.dram_tensor("oD", [NB, 4, IB], F32, kind="ExternalOutput")

    with _PatchedTileContext(nc) as tc:
        with (
            tc.tile_pool(name="const", bufs=1) as const_pool,
            tc.tile_pool(name="persist", bufs=1) as persist,
            tc.tile_pool(name="attn", bufs=ATTN_BUFS) as attn_pool,
            tc.tile_pool(name="osb", bufs=2) as osb_pool,
            tc.tile_pool(name="dsb", bufs=2) as dsb_pool,
            tc.tile_pool(name="dots", bufs=DOTS_BUFS, space="PSUM") as dots_ps,
            tc.tile_pool(name="accb", bufs=1, space="PSUM") as acc_ps,
            tc.tile_pool(name="denb", bufs=1, space="PSUM") as den_ps,
        ):
            # dummy exp: hoists the ~2.7us ACT exp-table load into the load
            # prologue, off the first real exp's critical path
            expwarm = const_pool.tile([1, 2], F32)
            nc.gpsimd.memset(expwarm, 0.0)
            nc.scalar.activation(
                out=expwarm[:, 0:1],
                in_=expwarm[:, 1:2],
                func=mybir.ActivationFunctionType.Exp,
            )
            ones = const_pool.tile([128, 1], BF16)
            nc.gpsimd.memset(ones, 1.0)
            wsrc = None
            if NWARM:
                wsrc = const_pool.tile([128, 512], BF16)
                nc.gpsimd.memset(wsrc, 1.0)

            qs = persist.tile([128, S], BF16)  # [h*64+d, s]
            ks = persist.tile([128, S], BF16)
            vsb = persist.tile([128, HPC * ST * 64], BF16)

            # ---- loads, sequenced for the block order [1, 3, 2, 0]: the
            # first block (ib1) needs k0/k1/q1/v; later q chunks trail in
            # first-use order across the three DMA-capable rings ----
            vv = vsb.rearrange("p (n t x) -> p n t x", n=HPC, x=64)

            def chunk(t, g):
                return t[:, g * 512 : (g + 1) * 512]

            nc.sync.dma_start(out=chunk(ks, 0), in_=chunk(kT, 0))
            nc.gpsimd.dma_start(out=chunk(ks, 1), in_=chunk(kT, 1))
            nc.scalar.dma_start(out=chunk(qs, 1), in_=chunk(qT, 1))
            nc.sync.dma_start(
                out=vv[:, 0, :, :],
                in_=v[0, :, :].rearrange("(t p) d -> p t d", p=128),
            )
            nc.gpsimd.dma_start(
                out=vv[:, 1, :, :],
                in_=v[1, :, :].rearrange("(t p) d -> p t d", p=128),
            )
            for g, eng in ((2, nc.sync), (3, nc.gpsimd)):
                eng.dma_start(out=chunk(ks, g), in_=chunk(kT, g))
            for g in (3, 2, 0):
                nc.scalar.dma_start(out=chunk(qs, g), in_=chunk(qT, g))

            if NWARM:
                # dummy-matmul burst during the load prologue: pokes the PE
                # HAM clock gate (1.2 -> 2.4 GHz needs ~3.4us of activity)
                wdst = dots_ps.tile([128, 1024], F32, tag="dots")
                for i in range(NWARM):
                    nc.tensor.matmul(
                        out=wdst[:, 0:512],
                        lhsT=wsrc[:, 0:128],
                        rhs=wsrc,
                        start=True,
                        stop=True,
                    )

            # ---- main: per (i-block, j-tile), both heads per PE slot via
            # array tiling; QK/exp stream runs SKEWJ ahead of AV/den,
            # across block boundaries (global software pipeline). ----
            state = {"act_ns": 0.0, "dve_ns": 0.0}

            def exp_cost(engine, cols):
                return (ACT_RATE * cols + ACT_OVH if engine == "act"
                        else DVE_RATE * cols + DVE_OVH)

            def route(cols, force=None):
                eng = force
                if eng is None:
                    eng = "act" if (
                        state["act_ns"] + exp_cost("act", cols)
                        <= state["dve_ns"] + exp_cost("dve", cols)
                    ) else "dve"
                state[eng + "_ns"] += exp_cost(eng, cols)
                return eng

            def emit_jtile(ib, jt):
                """Row-tiled QK pair (both heads) + merged exp (+masks)."""
                dk = jt - JPB * ib
                c = 0 if dk < 0 else dk * 128  # exact causal col start
                w = IB - c
                dots = dots_ps.tile([128, 1024], F32, tag="dots")
                at = attn_pool.tile([128, 1024], BF16, tag="at")
                for h in range(HPC):
                    r = slice(h * 64, (h + 1) * 64)
                    # h0 -> array rows 0:64 / bank 0, h1 -> rows 64:128 /
                    # bank 1: the two matmuls execute concurrently
                    nc.tensor.matmul(
                        out=dots[:, h * IB + c : (h + 1) * IB],
                        lhsT=ks[r, jt * 128 : (jt + 1) * 128],
                        rhs=qs[r, ib * IB + c : (ib + 1) * IB],
                        start=True,
                        stop=True,
                    )
                dview = dots.rearrange("p (n x) -> p n x", n=HPC)
                aview = at.rearrange("p (n x) -> p n x", n=HPC)
                eng = route(HPC * w, force="act" if dk >= 0 else None)
                if eng == "act" or not fastexp:
                    nc.scalar.activation(
                        out=aview[:, :, c:IB],
                        in_=dview[:, :, c:IB],
                        func=mybir.ActivationFunctionType.Exp,
                        scale=SCALE,
                    )
                else:
                    nc.vector.tensor_scalar(
                        out=aview.bitcast(I16)[:, :, c:IB],
                        in0=dview[:, :, c:IB],
                        scalar1=float(EXP_A),
                        scalar2=float(EXP_B),
                        op0=mybir.AluOpType.mult,
                        op1=mybir.AluOpType.add,
                    )
                if dk >= 0:
                    for h in range(HPC):
                        nc.gpsimd.affine_select(
                            out=at[:, h * IB + c : h * IB + c + 128],
                            in_=at[:, h * IB + c : h * IB + c + 128],
                            compare_op=mybir.AluOpType.is_ge,
                            fill=0.0,
                            base=0,
                            pattern=[[1, 128]],
                            channel_multiplier=-1,
                        )
                return (jt, at, c)

            def emit_av(ib, acc, njt, item):
                jt, at, c = item
                for h in range(HPC):
                    # col-tiled pair: v_h0 -> acc[0:64], v_h1 -> acc[64:128]
                    nc.tensor.matmul(
                        out=acc[h * 64 : (h + 1) * 64, c:IB],
                        lhsT=vsb[:, (h * ST + jt) * 64 : (h * ST + jt + 1) * 64],
                        rhs=at[:, h * IB + c : (h + 1) * IB],
                        start=(jt == 0),
                        stop=(jt == njt - 1),
                    )

            def emit_den(ib, den, njt, itemA, itemB):
                # 4-way col-tiled ones-matmuls: rows 0/32 <- h0/h1 of tile A,
                # rows 64/96 <- h0/h1 of tile B; all four run concurrently
                for slot, (jt, at, c) in enumerate((itemA, itemB)):
                    if jt is None:
                        continue
                    for h in range(HPC):
                        p = slot * 64 + h * 32
                        nc.tensor.matmul(
                            out=den[p : p + 1, c:IB],
                            lhsT=ones,
                            rhs=at[:, h * IB + c : (h + 1) * IB],
                            start=(jt < 2),
                            stop=(jt >= njt - 2),
                            tile_position=(0, p),
                        )

            RINGS = None  # set inside build (engines)

            def emit_num_chunk(ib, acc, sl, ring):
                outsb = osb_pool.tile([128, IB], BF16, tag="outsb")
                eng = route(sl.stop - sl.start)
                if eng == "dve":
                    nc.vector.tensor_copy(out=outsb[:, sl], in_=acc[:, sl])
                else:
                    nc.scalar.activation(
                        out=outsb[:, sl], in_=acc[:, sl],
                        func=mybir.ActivationFunctionType.Copy,
                    )
                ring.dma_start(out=oN[ib, :, sl], in_=outsb[:, sl])

            def emit_den_out(ib, den):
                dsb = dsb_pool.tile([97, IB], F32, tag="dsb")
                eng = route(IB)
                if eng == "dve":
                    nc.vector.tensor_copy(out=dsb, in_=den[0:97, :])
                else:
                    nc.scalar.activation(
                        out=dsb, in_=den[0:97, :],
                        func=mybir.ActivationFunctionType.Copy,
                    )
                for r in range(4):  # only rows 0/32/64/96 carry denominators
                    nc.sync.dma_start(
                        out=oD[ib, r], in_=dsb[32 * r : 32 * r + 1, :]
                    )

            ib_order = [int(x) for x in
                        os.environ.get("K_ORDER", "1,3,2,0").split(",")]
            qk_stream = [
                (ib, jt, JPB * (ib + 1)) for ib in ib_order
                for jt in range(JPB * (ib + 1))
            ]
            accs, dens = {}, {}
            inflight = []
            pend_den = {}
            rings = (nc.sync, nc.scalar, nc.gpsimd)

            def retire(ib, jt, njt, item, last):
                emit_av(ib, accs[ib], njt, item)
                if jt % 2 == 0:
                    pend_den[ib] = item
                else:
                    emit_den(ib, dens[ib], njt, pend_den.pop(ib), item)
                if jt == njt - 1:
                    acc, den = accs.pop(ib), dens.pop(ib)
                    if last:
                        for cch in range(4):
                            sl = slice(cch * 128, (cch + 1) * 128)
                            emit_num_chunk(ib, acc, sl, rings[cch % 3])
                    else:
                        emit_num_chunk(ib, acc, slice(0, IB), nc.sync)
                    emit_den_out(ib, den)

            nlast = qk_stream[-1][0]
            for ib, jt, njt in qk_stream:
                if jt == 0:
                    accs[ib] = acc_ps.tile([128, IB], F32, tag="acc",
                                           name=f"acc_{ib}")
                    dens[ib] = den_ps.tile([128, IB], F32, tag="den",
                                           name=f"den_{ib}")
                inflight.append((ib, jt, njt, emit_jtile(ib, jt)))
                if len(inflight) > SKEWJ:
                    pib, pjt, pnjt, item = inflight.pop(0)
                    retire(pib, pjt, pnjt, item, pib == nlast)
            for pib, pjt, pnjt, item in inflight:
                retire(pib, pjt, pnjt, item, pib == nlast)

    return nc


_NC_CACHE = {}


def _get_nc(key=True):
    if key not in _NC_CACHE:
        _NC_CACHE[key] = build_nc(fastexp=key)
    return _NC_CACHE[key]


def run(q, k, v, trace=False, fastexp=True, **kwargs):
    import ml_dtypes
    from concourse.bass_utils import run_bass_kernel_spmd

    kwargs.pop("mm_dtype", None)  # legacy knob from the v1 kernel's test.py
    nc = _get_nc(fastexp)
    bf = ml_dtypes.bfloat16
    q = np.ascontiguousarray(np.asarray(q), dtype=np.float32).reshape(H, S, D)
    k = np.ascontiguousarray(np.asarray(k), dtype=np.float32).reshape(H, S, D)
    v = np.ascontiguousarray(np.asarray(v), dtype=np.float32).reshape(H, S, D)
    qT = np.ascontiguousarray(q.transpose(0, 2, 1)).astype(bf)  # [H, 64, S]
    kT = np.ascontiguousarray(k.transpose(0, 2, 1)).astype(bf)
    v16 = v.astype(bf)
    in_maps = [
        {
            "qT": np.ascontiguousarray(qT[c * HPC : (c + 1) * HPC]).reshape(128, S),
            "kT": np.ascontiguousarray(kT[c * HPC : (c + 1) * HPC]).reshape(128, S),
            "v": np.ascontiguousarray(v16[c * HPC : (c + 1) * HPC]),
        }
        for c in range(NCORES)
    ]
    res = run_bass_kernel_spmd(
        nc, in_maps, core_ids=list(range(NCORES)), trace=trace, **kwargs
    )
    outs = []
    for c in range(NCORES):
        num = np.asarray(res.results[c]["oN"]).astype(np.float32)  # [NB, 128, IB]
        dd = np.array(res.results[c]["oD"], dtype=np.float32)  # [NB, 4, IB]
        # block 0's odd-j-tile den rows are never written for i<128 (the
        # first odd j-tile starts at the causal edge c=128): that PSUM
        # region is uninitialized garbage, and the true contribution is 0
        dd[0, 2:4, 0:128] = 0.0
        # num rows: 0:64 = h0 out^T, 64:128 = h1 out^T
        # den rows: [0]=h0-even-jts [1]=h1-even [2]=h0-odd [3]=h1-odd
        den_h = np.stack([dd[:, 0] + dd[:, 2], dd[:, 1] + dd[:, 3]])  # [2, NB, IB]
        num_h = np.stack([num[:, 0:64], num[:, 64:128]])  # [2, NB, 64, IB]
        res_h = num_h / den_h[:, :, None, :]
        outs.append(res_h.transpose(0, 1, 3, 2).reshape(HPC, S, D))
    out = np.concatenate(outs, axis=0)
    return out.reshape(B, H, S, D), res


def kernel(q, k, v):
    out, _ = run(q, k, v)
    return out


# revision 31
# speedup vs baseline: 1.1358x; 1.1358x over previous
"""Causal attention (B=1, H=16, S=2048, D=64, fp32 in/out) on 8 trn2 cores.

Sharding: 2 heads per core (fully head-parallel); inputs split / outputs
concatenated on host.

v5 design — PE-array tiling packs both heads into every matmul slot:
  - host hands each core qT/kT bf16 [128, S] pre-transposed (rows h*64+d)
    and v bf16 [HPC, S, D]; no on-chip transposes or casts;
  - QK, per (i-block 512, j-tile 128): TWO row-tiled matmuls run
    CONCURRENTLY on the PE (h0 in array rows 0:64, h1 in rows 64:128 —
    K=64 each), writing dots_h0/dots_h1 to the two banks of a
    [128, 1024] PSUM tile: one N-cycle slot computes both heads;
  - exp: ONE instruction per j-tile covers both heads via a 3-D AP over
    the two banks; routed ACT (exact, scale folded) vs DVE (Schraudolph
    fast-exp: bf16_bits = int16(dots*A + B), single fused tensor_scalar)
    by a greedy balancer; diagonal tiles always exact + gpsimd
    affine_select causal zeroing (end-to-end rel_err ~5e-3 vs 2e-2 budget);
  - AV, per j-tile: TWO col-tiled matmuls run concurrently (v_h0 ->
    acc[0:64], v_h1 -> acc[64:128], M=64 each, K=128): one N-cycle slot;
    the softmax denominators (the ones-column of older revisions doesn't
    fit col tiling) come from 4-way col-tiled ones-matmuls (M=1 at
    partitions 0/32/64/96, two j-tiles x two heads per slot);
  - epilogue: acc + den PSUM banks copied raw to SBUF and DMA'd out
    UNNORMALIZED; the host transposes and divides in numpy.
PE work per core: QK 17.4K + AV 17.4K + den 8.7K ~= 44K cycles; ACT/DVE
carry ~10us of exp each; everything else hides under the PE stream.
"""

import os

import numpy as np

import concourse.bass as bass
import concourse.mybir as mybir
import concourse.tile as tile
from concourse.vector_clock import ScopedClock

B, H, S, D = 1, 16, 2048, 64
NCORES = 8
HPC = H // NCORES  # heads per core
ST = S // 128  # seq tiles of 128
IB = 512  # i-block width
NB = S // IB  # i-blocks
JPB = IB // 128  # j-tiles per i-block (4)
SCALE = float(D) ** -0.5

F32 = mybir.dt.float32
BF16 = mybir.dt.bfloat16
I16 = mybir.dt.int16

# Schraudolph fast-exp constants (bf16-bits variant, scale folded in):
#   bf16_bits(exp(scale*x)) ~= int16(x * EXP_A + EXP_B)
EXP_C = 330000.0  # sawtooth-centering offset (tuned end-to-end)
EXP_A = SCALE * (2.0**23 / np.log(2.0)) / 65536.0
EXP_B = (127.0 * 2.0**23 - EXP_C) / 65536.0 + 0.25  # +0.25: round/trunc-robust

# greedy exp-router cost model (ns): per-column rate + per-instruction setup
ACT_RATE, ACT_OVH = 0.833, 280.0
DVE_RATE, DVE_OVH = 1.042, 170.0


# --------------------------------------------------------------------------
# Workarounds for the walrus in this container: an instruction may carry at
# most ONE sync-wait command ("Too many sync wait commands" in setupSyncWait
# otherwise).  (a) split the TileContext final drain into one drain per
# semaphore, (b) split any scheduled instruction with >1 wait by hoisting
# extra waits onto preceding same-engine NoOps.
# --------------------------------------------------------------------------
_MAXW = 1


def _split_drain_and_barrier(self, tick_clock, wait_clock):
    vclock = tick_clock.global_clock
    pending = [(proc, vclock[proc]) for proc in range(len(vclock)) if vclock[proc] > 0]
    engines = [self.nc.sync, self.nc.vector, self.nc.scalar, self.nc.gpsimd,
               self.nc.tensor]
    for i in range(0, len(pending), _MAXW):
        d = engines[(i // _MAXW) % len(engines)].drain()
        sc = ScopedClock()
        for proc, t in pending[i : i + _MAXW]:
            sc.require_at_least(None, proc, t)
        wait_clock.add_sem_waits(d.ins, sc)
    self.nc.all_engine_barrier()
    popped = self.nc._tile_sem_poison_stack.pop()
    assert popped is self._sem_poison
    self.nc.clear_and_free_semaphores(list(self.sems.allocated().values()))
    self.nc.all_engine_barrier()


_orig_lower = tile.TileContext._lower_ordered_insts


def _split_waits_lower(self, ordered):
    import bass_rust

    for bbname in list(ordered.keys()):
        out = []
        for inst in ordered[bbname]:
            si = inst.sync_info
            if si is not None and len(si.on_wait) > _MAXW:
                waits = list(si.on_wait)
                extra, keep = waits[:-_MAXW], waits[-_MAXW:]
                for i in range(0, len(extra), _MAXW):
                    nop = mybir.InstNoOp(
                        name=f"{inst.name}-wsplit{i}", ins=[], outs=[]
                    )
                    nop.engine = inst.engine
                    nop.sync_info = bass_rust.SyncInfo(
                        on_wait=extra[i : i + _MAXW], on_update=[]
                    )
                    out.append(nop)
                inst.sync_info = bass_rust.SyncInfo(
                    on_wait=keep, on_update=list(si.on_update)
                )
            out.append(inst)
        ordered[bbname] = out
    return _orig_lower(self, ordered)


class _PatchedTileContext(tile.TileContext):
    _drain_and_barrier = _split_drain_and_barrier
    _lower_ordered_insts = _split_waits_lower


# --------------------------------------------------------------------------
# Kernel build
# --------------------------------------------------------------------------


def build_nc(fastexp=True):
    SKEWJ = int(os.environ.get("K_SKEWJ", "3"))  # j-tile QK->AV lookahead
    DOTS_BUFS = int(os.environ.get("K_DOTS", "3"))
    ATTN_BUFS = int(os.environ.get("K_ATTN", "8"))
    NWARM = int(os.environ.get("K_WARM", "0"))
    if os.environ.get("K_FASTEXP", "1") == "0":
        fastexp = False

    nc = bass.Bass("TRN2")
    qT = nc.dram_tensor("qT", [128, S], BF16, kind="ExternalInput")
    kT = nc.dram_tensor("kT", [128, S], BF16, kind="ExternalInput")
    v = nc.dram_tensor("v", [HPC, S, D], BF16, kind="ExternalInput")
    oN = nc.dram_tensor("oN", [NB, 128, IB], BF16, kind="ExternalOutput")
    oD = nc.dram_tensor("oD", [NB, 4, IB], F32, kind="ExternalOutput")

    with _PatchedTileContext(nc) as tc:
        with (
            tc.tile_pool(name="const", bufs=1) as const_pool,
            tc.tile_pool(name="persist", bufs=1) as persist,
            tc.tile_pool(name="attn", bufs=ATTN_BUFS) as attn_pool,
            tc.tile_pool(name="osb", bufs=2) as osb_pool,
            tc.tile_pool(name="dsb", bufs=2) as dsb_pool,
            tc.tile_pool(name="dots", bufs=DOTS_BUFS, space="PSUM") as dots_ps,
            tc.tile_pool(name="accb", bufs=1, space="PSUM") as acc_ps,
            tc.tile_pool(name="denb", bufs=1, space="PSUM") as den_ps,
        ):
            # dummy exp: hoists the ~2.7us ACT exp-table load into the load
            # prologue, off the first real exp's critical path
            expwarm = const_pool.tile([1, 2], F32)
            nc.gpsimd.memset(expwarm, 0.0)
            nc.scalar.activation(
                out=expwarm[:, 0:1],
                in_=expwarm[:, 1:2],
                func=mybir.ActivationFunctionType.Exp,
            )
            ones = const_pool.tile([128, 1], BF16)
            nc.gpsimd.memset(ones, 1.0)
            wsrc = None
            if NWARM:
                wsrc = const_pool.tile([128, 512], BF16)
                nc.gpsimd.memset(wsrc, 1.0)

            qs = persist.tile([128, S], BF16)  # [h*64+d, s]
            ks = persist.tile([128, S], BF16)
            vsb = persist.tile([128, HPC * ST * 64], BF16)

            # ---- loads: k0/q0/v first (they gate the pipeline start),
            # split across the three DMA-capable rings; the rest trails ----
            vv = vsb.rearrange("p (n t x) -> p n t x", n=HPC, x=64)
            s0 = slice(0, 512)
            nc.sync.dma_start(out=ks[:, s0], in_=kT[:, s0])
            nc.scalar.dma_start(out=qs[0:64, s0], in_=qT[0:64, s0])
            nc.gpsimd.dma_start(out=qs[64:128, s0], in_=qT[64:128, s0])
            nc.sync.dma_start(
                out=vv[:, 0, :, :],
                in_=v[0, :, :].rearrange("(t p) d -> p t d", p=128),
            )
            nc.gpsimd.dma_start(
                out=vv[:, 1, :, :],
                in_=v[1, :, :].rearrange("(t p) d -> p t d", p=128),
            )
            for g in range(1, 4):
                sl = slice(g * 512, (g + 1) * 512)
                nc.sync.dma_start(out=ks[:, sl], in_=kT[:, sl])
                nc.scalar.dma_start(out=qs[:, sl], in_=qT[:, sl])

            if NWARM:
                # dummy-matmul burst during the load prologue: pokes the PE
                # HAM clock gate (1.2 -> 2.4 GHz needs ~3.4us of activity)
                wdst = dots_ps.tile([128, 1024], F32, tag="dots")
                for i in range(NWARM):
                    nc.tensor.matmul(
                        out=wdst[:, 0:512],
                        lhsT=wsrc[:, 0:128],
                        rhs=wsrc,
                        start=True,
                        stop=True,
                    )

            # ---- main: per (i-block, j-tile), both heads per PE slot via
            # array tiling; QK/exp stream runs SKEWJ ahead of AV/den,
            # across block boundaries (global software pipeline). ----
            state = {"act_ns": 0.0, "dve_ns": 0.0}

            def exp_cost(engine, cols):
                return (ACT_RATE * cols + ACT_OVH if engine == "act"
                        else DVE_RATE * cols + DVE_OVH)

            def route(cols, force=None):
                eng = force
                if eng is None:
                    eng = "act" if (
                        state["act_ns"] + exp_cost("act", cols)
                        <= state["dve_ns"] + exp_cost("dve", cols)
                    ) else "dve"
                state[eng + "_ns"] += exp_cost(eng, cols)
                return eng

            def emit_jtile(ib, jt):
                """Row-tiled QK pair (both heads) + merged exp (+masks)."""
                dk = jt - JPB * ib
                c = 0 if dk < 0 else dk * 128  # exact causal col start
                w = IB - c
                dots = dots_ps.tile([128, 1024], F32, tag="dots")
                at = attn_pool.tile([128, 1024], BF16, tag="at")
                for h in range(HPC):
                    r = slice(h * 64, (h + 1) * 64)
                    # h0 -> array rows 0:64 / bank 0, h1 -> rows 64:128 /
                    # bank 1: the two matmuls execute concurrently
                    nc.tensor.matmul(
                        out=dots[:, h * IB + c : (h + 1) * IB],
                        lhsT=ks[r, jt * 128 : (jt + 1) * 128],
                        rhs=qs[r, ib * IB + c : (ib + 1) * IB],
                        start=True,
                        stop=True,
                    )
                dview = dots.rearrange("p (n x) -> p n x", n=HPC)
                aview = at.rearrange("p (n x) -> p n x", n=HPC)
                eng = route(HPC * w, force="act" if dk >= 0 else None)
                if eng == "act" or not fastexp:
                    nc.scalar.activation(
                        out=aview[:, :, c:IB],
                        in_=dview[:, :, c:IB],
                        func=mybir.ActivationFunctionType.Exp,
                        scale=SCALE,
                    )
                else:
                    nc.vector.tensor_scalar(
                        out=aview.bitcast(I16)[:, :, c:IB],
                        in0=dview[:, :, c:IB],
                        scalar1=float(EXP_A),
                        scalar2=float(EXP_B),
                        op0=mybir.AluOpType.mult,
                        op1=mybir.AluOpType.add,
                    )
                if dk >= 0:
                    for h in range(HPC):
                        nc.gpsimd.affine_select(
                            out=at[:, h * IB + c : h * IB + c + 128],
                            in_=at[:, h * IB + c : h * IB + c + 128],
                            compare_op=mybir.AluOpType.is_ge,
                            fill=0.0,
                            base=0,
                            pattern=[[1, 128]],
                            channel_multiplier=-1,
                        )
                return (jt, at, c)

            def emit_av(ib, acc, njt, item):
                jt, at, c = item
                for h in range(HPC):
                    # col-tiled pair: v_h0 -> acc[0:64], v_h1 -> acc[64:128]
                    nc.tensor.matmul(
                        out=acc[h * 64 : (h + 1) * 64, c:IB],
                        lhsT=vsb[:, (h * ST + jt) * 64 : (h * ST + jt + 1) * 64],
                        rhs=at[:, h * IB + c : (h + 1) * IB],
                        start=(jt == 0),
                        stop=(jt == njt - 1),
                    )

            def emit_den(ib, den, njt, itemA, itemB):
                # 4-way col-tiled ones-matmuls: rows 0/32 <- h0/h1 of tile A,
                # rows 64/96 <- h0/h1 of tile B; all four run concurrently
                for slot, (jt, at, c) in enumerate((itemA, itemB)):
                    if jt is None:
                        continue
                    for h in range(HPC):
                        p = slot * 64 + h * 32
                        nc.tensor.matmul(
                            out=den[p : p + 1, c:IB],
                            lhsT=ones,
                            rhs=at[:, h * IB + c : (h + 1) * IB],
                            start=(jt < 2),
                            stop=(jt >= njt - 2),
                            tile_position=(0, p),
                        )

            RINGS = None  # set inside build (engines)

            def emit_num_chunk(ib, acc, sl, ring):
                outsb = osb_pool.tile([128, IB], BF16, tag="outsb")
                eng = route(sl.stop - sl.start)
                if eng == "dve":
                    nc.vector.tensor_copy(out=outsb[:, sl], in_=acc[:, sl])
                else:
                    nc.scalar.activation(
                        out=outsb[:, sl], in_=acc[:, sl],
                        func=mybir.ActivationFunctionType.Copy,
                    )
                ring.dma_start(out=oN[ib, :, sl], in_=outsb[:, sl])

            def emit_den_out(ib, den):
                dsb = dsb_pool.tile([97, IB], F32, tag="dsb")
                eng = route(IB)
                if eng == "dve":
                    nc.vector.tensor_copy(out=dsb, in_=den[0:97, :])
                else:
                    nc.scalar.activation(
                        out=dsb, in_=den[0:97, :],
                        func=mybir.ActivationFunctionType.Copy,
                    )
                for r in range(4):  # only rows 0/32/64/96 carry denominators
                    nc.sync.dma_start(
                        out=oD[ib, r], in_=dsb[32 * r : 32 * r + 1, :]
                    )

            ib_order = [int(x) for x in
                        os.environ.get("K_ORDER", "0,1,2,3").split(",")]
            qk_stream = [
                (ib, jt, JPB * (ib + 1)) for ib in ib_order
                for jt in range(JPB * (ib + 1))
            ]
            accs, dens = {}, {}
            inflight = []
            pend_den = {}
            rings = (nc.sync, nc.scalar, nc.gpsimd)

            def retire(ib, jt, njt, item, last):
                emit_av(ib, accs[ib], njt, item)
                if jt % 2 == 0:
                    pend_den[ib] = item
                else:
                    emit_den(ib, dens[ib], njt, pend_den.pop(ib), item)
                if jt == njt - 1:
                    acc, den = accs.pop(ib), dens.pop(ib)
                    if last:
                        for cch in range(4):
                            sl = slice(cch * 128, (cch + 1) * 128)
                            emit_num_chunk(ib, acc, sl, rings[cch % 3])
                    else:
                        emit_num_chunk(ib, acc, slice(0, IB), nc.sync)
                    emit_den_out(ib, den)

            nlast = qk_stream[-1][0]
            for ib, jt, njt in qk_stream:
                if jt == 0:
                    accs[ib] = acc_ps.tile([128, IB], F32, tag="acc",
                                           name=f"acc_{ib}")
                    dens[ib] = den_ps.tile([128, IB], F32, tag="den",
                                           name=f"den_{ib}")
                inflight.append((ib, jt, njt, emit_jtile(ib, jt)))
                if len(inflight) > SKEWJ:
                    pib, pjt, pnjt, item = inflight.pop(0)
                    retire(pib, pjt, pnjt, item, pib == nlast)
            for pib, pjt, pnjt, item in inflight:
                retire(pib, pjt, pnjt, item, pib == nlast)

    return nc


_NC_CACHE = {}


def _get_nc(key=True):
    if key not in _NC_CACHE:
        _NC_CACHE[key] = build_nc(fastexp=key)
    return _NC_CACHE[key]


def run(q, k, v, trace=False, fastexp=True, **kwargs):
    import ml_dtypes
    from concourse.bass_utils import run_bass_kernel_spmd

    kwargs.pop("mm_dtype", None)  # legacy knob from the v1 kernel's test.py
    nc = _get_nc(fastexp)
    bf = ml_dtypes.bfloat16
    q = np.ascontiguousarray(np.asarray(q), dtype=np.float32).reshape(H, S, D)
    k = np.ascontiguousarray(np.asarray(k), dtype=np.float32).reshape(H, S, D)
    v = np.ascontiguousarray(np.asarray(v), dtype=np.float32).reshape(H, S, D)
    qT = np.ascontiguousarray(q.transpose(0, 2, 1)).astype(bf)  # [H, 64, S]
    kT = np.ascontiguousarray(k.transpose(0, 2, 1)).astype(bf)
    v16 = v.astype(bf)
    in_maps = [
        {
            "qT": np.ascontiguousarray(qT[c * HPC : (c + 1) * HPC]).reshape(128, S),
            "kT": np.ascontiguousarray(kT[c * HPC : (c + 1) * HPC]).reshape(128, S),
            "v": np.ascontiguousarray(v16[c * HPC : (c + 1) * HPC]),
        }
        for c in range(NCORES)
    ]
    res = run_bass_kernel_spmd(
        nc, in_maps, core_ids=list(range(NCORES)), trace=trace, **kwargs
    )
    outs = []
    for c in range(NCORES):
        num = np.asarray(res.results[c]["oN"]).astype(np.float32)  # [NB, 128, IB]
        dd = np.array(res.results[c]["oD"], dtype=np.float32)  # [NB, 4, IB]
        # block 0's odd-j-tile den rows are never written for i<128 (the
        # first odd j-tile starts at the causal edge c=128): that PSUM
        # region is uninitialized garbage, and the true contribution is 0
        dd[0, 2:4, 0:128] = 0.0
        # num rows: 0:64 = h0 out^T, 64:128 = h1 out^T
        # den rows: [0]=h0-even-jts [1]=h1-even [2]=h0-odd [3]=h1-odd
        den_h = np.stack([dd[:, 0] + dd[:, 2], dd[:, 1] + dd[:, 3]])  # [2, NB, IB]
        num_h = np.stack([num[:, 0:64], num[:, 64:128]])  # [2, NB, 64, IB]
        res_h = num_h / den_h[:, :, None, :]
        outs.append(res_h.transpose(0, 1, 3, 2).reshape(HPC, S, D))
    out = np.concatenate(outs, axis=0)
    return out.reshape(B, H, S, D), res


def kernel(q, k, v):
    out, _ = run(q, k, v)
    return out
